# revision 6
# baseline (speedup 1.0000x reference)
"""CRF negative-log-likelihood loss on 8 Trainium2 NeuronCores.

Strategy: data-parallel over batch (128 sequences per core). The forward
(log-partition) recurrence runs on device in the exp domain:

    W_{t+1} = (E' @ W_t) * exp(logits_t - C0),   E' = exp(transitions)

All matmuls are bf16 (1 PE cycle/row vs 4 for fp32). Two batch halves are
stacked block-diagonally on the partition axis (2x52 = 104 contract rows,
106 output rows: 2x52 W' blocks + 2 q rows from r = exp(transitions[STOP])),
so one matmul + one DVE multiply advance 2 batch elements per column.
Three independent chains (22/21/21 columns) hide the PE->DVE->PE latency.
With C0 = 4.9 the state drift over 512 steps stays within fp32/bf16 range,
so no rescaling is needed; the host reconstructs the log-partition as
ln q_t + C0*t and selects t = lens[b]. q rows are copied out of PSUM by the
(otherwise idle) Activation engine and DMA'd per 128-step strip. Gold-path
scores are cheap host-side gathers.
"""

import numpy as np
import ml_dtypes

BF16 = ml_dtypes.bfloat16

# problem constants (hardcoded per contract)
B, T, K = 1024, 512, 52
START, STOP = 50, 51
NCORES = 8
BPC = B // NCORES          # 128 sequences per core
HALF = 64                  # stacked pair columns per core
C0 = 4.9                   # per-step constant log-shift folded into exp(logits)
KK = 2 * K                 # 104 contract rows
MO = KK + 2                # 106 matmul output rows (104 W' + 2 q)
QROW = 96                  # legal AP start partition covering q rows 104..105
QNR = MO - QROW            # 10 rows in the q copy
CH = 64                    # time steps per ex DMA chunk
QB = 8                     # steps per PSUM tile (one 2KB PSUM bank)
NS = 128                   # steps per q strip
NBLK = (T + 1 + NS - 1) // NS   # 5 strips (t = 0..512)
CHAINS = [(0, 64)]         # (col offset, width): single chain, all columns

_PROG_CACHE = {}


def _build_program():
    import concourse.mybir as mybir
    import concourse.tile as tile
    from concourse import bacc

    f32 = mybir.dt.float32
    bf16 = mybir.dt.bfloat16

    nc = bacc.Bacc("TRN2", target_bir_lowering=False, debug=False,
                   num_devices=NCORES)
    exL_d = nc.dram_tensor("exL", [KK, T, HALF], bf16, kind="ExternalInput")
    ehatT_d = nc.dram_tensor("ehatT", [KK, MO], bf16, kind="ExternalInput")
    winit_d = nc.dram_tensor("winit", [KK, HALF], bf16, kind="ExternalInput")
    qout_d = [nc.dram_tensor(f"qout{g}", [NBLK, QNR, NS * wg], f32,
                             kind="ExternalOutput")
              for g, (_, wg) in enumerate(CHAINS)]

    with tile.TileContext(nc) as tc:
        with (
            tc.tile_pool(name="const", bufs=1) as cpool,
            tc.tile_pool(name="ex", bufs=2) as expool,
            tc.tile_pool(name="w", bufs=6) as wpool,
            tc.tile_pool(name="qs", bufs=2) as qspool,
            tc.tile_pool(name="u", bufs=4, space="PSUM") as ppool,
        ):
            ehatT = cpool.tile([KK, MO], bf16)
            nc.sync.dma_start(ehatT[:], ehatT_d[:])

            W = []
            for g, (lo, wg) in enumerate(CHAINS):
                w0 = wpool.tile([KK, wg], bf16, tag=f"w{g}", name=f"w0_{g}")
                nc.sync.dma_start(w0[:], winit_d[:, lo:lo + wg])
                W.append(w0)

            ex = None
            ua = [None] * len(CHAINS)
            qsb = [None] * len(CHAINS)
            for t in range(T + 1):
                if t % CH == 0 and t < T:
                    ex = expool.tile([KK, CH, HALF], bf16, tag="ex", name="ex")
                    nc.sync.dma_start(ex[:], exL_d[:, t:t + CH, :])
                i = t % QB
                for g, (lo, wg) in enumerate(CHAINS):
                    if t % NS == 0:
                        qsb[g] = qspool.tile([QNR, NS * wg], f32,
                                             tag=f"qs{g}", name=f"qs{g}")
                    if i == 0:
                        ua[g] = ppool.tile([MO, QB * wg], f32,
                                           tag=f"u{g}", name=f"u{g}")
                    u = ua[g]
                    # u[:, i] = [E'@W0 ; E'@W1 ; q0 ; q1]  (one bf16 matmul)
                    nc.tensor.matmul(u[:, i * wg:(i + 1) * wg], ehatT[:],
                                     W[g][:], start=True, stop=True)
                    if t < T:
                        wn = wpool.tile([KK, wg], bf16, tag=f"w{g}",
                                        name=f"wn{g}")
                        nc.vector.tensor_mul(
                            wn[:], u[0:KK, i * wg:(i + 1) * wg],
                            ex[:, t % CH, lo:lo + wg])
                        W[g] = wn
                    if i == QB - 1 or t == T:
                        # flush q rows (plus the 96..103 pad forced by the
                        # AP start-partition rule) to the SBUF strip
                        n = (i + 1) * wg
                        off = (t - i) % NS * wg
                        nc.scalar.copy(qsb[g][:, off:off + n],
                                       u[QROW:MO, 0:n])
                    if t % NS == NS - 1 or t == T:
                        nc.sync.dma_start(qout_d[g][t // NS], qsb[g][:])

    _dedup_ldweights(nc)
    nc.compile()
    return nc


def _dedup_ldweights(nc):
    """Drop redundant stationary reloads. Every matmul shares the one ehatT
    stationary, but bass emits an InstLdweights per matmul. Keep only the
    loads whose matmul starts a fresh PSUM tile (those matmuls can carry a
    second semaphore wait, which Bacc.move_matmul_waits_to_ldweights moves
    onto the immediately preceding load — so that load must stay put);
    delete the rest, leaving the PE stationary resident across steps."""
    import concourse.mybir as mybir

    for blk in nc.main_func.blocks:
        insts = blk.instructions
        out = []
        mm_idx = 0
        k = 0
        while k < len(insts):
            inst = insts[k]
            if (isinstance(inst, mybir.InstLdweights)
                    and k + 1 < len(insts)
                    and isinstance(insts[k + 1], mybir.InstMatmult)):
                t, g = mm_idx // len(CHAINS), mm_idx % len(CHAINS)
                mm_idx += 1
                si = inst.sync_info
                clean = si is None or (not si.on_wait and not si.on_update)
                if t % QB != 0 and clean:
                    out.append(insts[k + 1])   # matmul only
                else:
                    out.append(inst)
                    out.append(insts[k + 1])
                k += 2
                continue
            out.append(inst)
            k += 1
        if mm_idx:
            blk.instructions[:] = out


def _get_program():
    if "p" not in _PROG_CACHE:
        _PROG_CACHE["p"] = _build_program()
    return _PROG_CACHE["p"]


def _host_prep(logits, trans):
    Ep = np.exp(trans.astype(np.float64)).astype(np.float32)   # [K,K]
    r = np.exp(trans[STOP].astype(np.float64)).astype(np.float32)
    eh = np.zeros((KK, MO), np.float32)        # [contract k, out row m]
    eh[0:K, 0:K] = Ep.T
    eh[K:KK, K:KK] = Ep.T
    eh[0:K, KK] = r
    eh[K:KK, KK + 1] = r
    ehatT = eh.astype(BF16)
    winit = np.zeros((KK, HALF), np.float32)
    winit[START] = 1.0
    winit[K + START] = 1.0
    winit = winit.astype(BF16)
    ex = np.exp(logits.astype(np.float32) - C0)               # [B,T,K]
    in_maps = []
    for c in range(NCORES):
        sh = ex[c * BPC:(c + 1) * BPC]                        # [128,T,K]
        b0 = sh[0:HALF].transpose(2, 1, 0)                    # [K,T,64]
        b1 = sh[HALF:BPC].transpose(2, 1, 0)
        exL = np.ascontiguousarray(
            np.concatenate([b0, b1], axis=0)).astype(BF16)    # [104,T,64]
        in_maps.append({"exL": exL, "ehatT": ehatT, "winit": winit})
    return in_maps


def _host_post(results, lens):
    """partition(t) = ln q_t + C0*t, selected at t = lens[b]."""
    partition = np.empty(B, np.float64)
    tt = np.arange(1, T + 1)
    for c in range(NCORES):
        for g, (lo, wg) in enumerate(CHAINS):
            qarr = results[c][f"qout{g}"].reshape(NBLK, QNR, NS, wg)
            q0 = qarr[:, 8].reshape(NBLK * NS, wg)[:T + 1]    # [513,wg]
            q1 = qarr[:, 9].reshape(NBLK * NS, wg)[:T + 1]
            for qs, boff in ((q0, 0), (q1, HALF)):
                part_at = np.log(qs[1:].astype(np.float64)) + C0 * tt[:, None]
                bidx = c * BPC + boff + lo + np.arange(wg)
                partition[bidx] = part_at[lens[bidx] - 1, np.arange(wg)]
    return partition


def _gold_scores(logits, trans, labels, lens):
    logits64 = logits.astype(np.float64)
    trans64 = trans.astype(np.float64)
    labels_ext = np.concatenate(
        [np.full((B, 1), START, np.int64), labels,
         np.full((B, 1), STOP, np.int64)], 1)
    pos = np.arange(T + 2)[None, :]
    labels_ext = np.where(pos < (lens + 1)[:, None], labels_ext, STOP)
    prev, nxt = labels_ext[:, :-1], labels_ext[:, 1:]
    m_trn = (np.arange(T + 1)[None, :] < (lens + 1)[:, None])
    transition_score = (trans64[nxt, prev] * m_trn).sum(1)
    em = np.take_along_axis(logits64, labels[:, :, None], 2)[:, :, 0]
    m_em = (np.arange(T)[None, :] < lens[:, None])
    emission_score = (em * m_em).sum(1)
    return emission_score, transition_score


def kernel(logits, transitions, labels, lens, _trace=False, **_kw):
    from concourse.bass_utils import run_bass_kernel_spmd

    logits = np.asarray(logits, dtype=np.float32)
    transitions = np.asarray(transitions, dtype=np.float32)
    labels_np = np.asarray(labels).astype(np.int64)
    lens_np = np.asarray(lens).astype(np.int64)

    nc = _get_program()
    in_maps = _host_prep(logits, transitions)
    out = run_bass_kernel_spmd(nc, in_maps, list(range(NCORES)),
                               trace=_trace)
    partition = _host_post(out.results, lens_np)
    emission, transition = _gold_scores(logits, transitions, labels_np,
                                        lens_np)
    loss = partition + emission - transition
    if _trace:
        kernel._last_exec_ns = out.exec_time_ns
        kernel._last_profile = out.profile_json
        kernel._last_out = out
    return loss.astype(np.float32)


# revision 12
# speedup vs baseline: 1.1502x; 1.1502x over previous
"""CRF negative-log-likelihood loss on 8 Trainium2 NeuronCores.

Strategy: data-parallel over batch (128 sequences per core). The forward
(log-partition) recurrence runs on device in the exp domain:

    W_{t+1} = (E' @ W_t) * exp(logits_t - C0),   E' = exp(transitions)

All matmuls are bf16 (1 PE cycle/row vs 4 for fp32). Two batch halves are
stacked block-diagonally on the partition axis (2x52 = 104 contract rows,
106 output rows: 2x52 W' blocks + 2 q rows from r = exp(transitions[STOP])),
so one matmul + one DVE multiply advance 2 batch elements per column.
Three independent chains (22/21/21 columns) hide the PE->DVE->PE latency.
With C0 = 4.9 the state drift over 512 steps stays within fp32/bf16 range,
so no rescaling is needed; the host reconstructs the log-partition as
ln q_t + C0*t and selects t = lens[b]. q rows are copied out of PSUM by the
(otherwise idle) Activation engine and DMA'd per 128-step strip. Gold-path
scores are cheap host-side gathers.
"""

import numpy as np
import ml_dtypes

BF16 = ml_dtypes.bfloat16

# problem constants (hardcoded per contract)
B, T, K = 1024, 512, 52
START, STOP = 50, 51
NCORES = 8
BPC = B // NCORES          # 128 sequences per core
HALF = 64                  # stacked pair columns per core
C0 = 4.9                   # per-step constant log-shift folded into exp(logits)
KK = 2 * K                 # 104 contract rows
MO = KK + 2                # 106 matmul output rows (104 W' + 2 q)
QROW = 96                  # legal AP start partition covering q rows 104..105
QNR = MO - QROW            # 10 rows in the q copy
CH = 64                    # time steps per ex DMA chunk
QB = 8                     # steps per PSUM tile (one 2KB PSUM bank)
NS = 128                   # steps per q strip
NBLK = (T + 1 + NS - 1) // NS   # 5 strips (t = 0..512)
CHAINS = [(0, 32), (32, 32)]   # (col offset, width)
JUNK = 2                   # PE warm-up matmuls per step (p-state hold)

_PROG_CACHE = {}


def _build_program():
    import concourse.mybir as mybir
    import concourse.tile as tile
    from concourse import bacc

    f32 = mybir.dt.float32
    bf16 = mybir.dt.bfloat16

    nc = bacc.Bacc("TRN2", target_bir_lowering=False, debug=False,
                   num_devices=NCORES)
    exL_d = nc.dram_tensor("exL", [KK, T, HALF], bf16, kind="ExternalInput")
    ehatT_d = nc.dram_tensor("ehatT", [KK, MO], bf16, kind="ExternalInput")
    winit_d = nc.dram_tensor("winit", [KK, HALF], bf16, kind="ExternalInput")
    qout_d = [nc.dram_tensor(f"qout{g}", [NBLK, QNR, NS * wg], f32,
                             kind="ExternalOutput")
              for g, (_, wg) in enumerate(CHAINS)]

    with tile.TileContext(nc) as tc:
        with (
            tc.tile_pool(name="const", bufs=1) as cpool,
            tc.tile_pool(name="ex", bufs=2) as expool,
            tc.tile_pool(name="w", bufs=6) as wpool,
            tc.tile_pool(name="qs", bufs=2) as qspool,
            tc.tile_pool(name="u", bufs=3, space="PSUM") as ppool,
            tc.tile_pool(name="uj", bufs=1, space="PSUM") as jpool,
        ):
            ehatT = cpool.tile([KK, MO], bf16)
            nc.sync.dma_start(ehatT[:], ehatT_d[:])
            wjunk = cpool.tile([KK, HALF], bf16)
            nc.sync.dma_start(wjunk[:], winit_d[:])
            ujunk = jpool.tile([MO, HALF], f32, tag="junk", name="ujunk")

            W = []
            for g, (lo, wg) in enumerate(CHAINS):
                w0 = wpool.tile([KK, wg], bf16, tag=f"w{g}", name=f"w0_{g}")
                nc.sync.dma_start(w0[:], winit_d[:, lo:lo + wg])
                W.append(w0)

            ex = None
            ua = [None] * len(CHAINS)
            qsb = [None] * len(CHAINS)
            for t in range(T + 1):
                if t % CH == 0 and t < T:
                    ex = expool.tile([KK, CH, HALF], bf16, tag="ex", name="ex")
                    nc.sync.dma_start(ex[:], exL_d[:, t:t + CH, :])
                i = t % QB
                for g, (lo, wg) in enumerate(CHAINS):
                    if t % NS == 0:
                        qsb[g] = qspool.tile([QNR, NS * wg], f32,
                                             tag=f"qs{g}", name=f"qs{g}")
                    if i == 0:
                        ua[g] = ppool.tile([MO, QB * wg], f32,
                                           tag=f"u{g}", name=f"u{g}")
                    u = ua[g]
                    # u[:, i] = [E'@W0 ; E'@W1 ; q0 ; q1]  (one bf16 matmul)
                    nc.tensor.matmul(u[:, i * wg:(i + 1) * wg], ehatT[:],
                                     W[g][:], start=True, stop=True)
                    if t < T:
                        wn = wpool.tile([KK, wg], bf16, tag=f"w{g}",
                                        name=f"wn{g}")
                        nc.vector.tensor_mul(
                            wn[:], u[0:KK, i * wg:(i + 1) * wg],
                            ex[:, t % CH, lo:lo + wg])
                        W[g] = wn
                    if i == QB - 1 or t == T:
                        # flush q rows (plus the 96..103 pad forced by the
                        # AP start-partition rule) to the SBUF strip
                        n = (i + 1) * wg
                        off = (t - i) % NS * wg
                        nc.scalar.copy(qsb[g][:, off:off + n],
                                       u[QROW:MO, 0:n])
                    if t % NS == NS - 1 or t == T:
                        nc.sync.dma_start(qout_d[g][t // NS], qsb[g][:])
                for _ in range(JUNK if t < T else 0):
                    # dependency-free warmers: keep the PE p-state up so the
                    # latency-critical matmuls run at full clock
                    nc.tensor.matmul(ujunk[:], ehatT[:], wjunk[:],
                                     start=True, stop=True)

    _strip_self_waits(nc)
    _dedup_ldweights(nc)
    nc.compile()
    return nc


def _strip_self_waits(nc):
    """Remove semaphore waits that an engine places on a semaphore updated
    only by that same engine: in-order execution already guarantees them
    (the original program would deadlock otherwise), and each one stalls
    the sequencer for the update-propagation delay of its own previous
    instruction. Standalone InstEventSemaphore carriers left with no waits
    are dropped entirely."""
    import concourse.mybir as mybir

    updaters = {}
    for blk in nc.main_func.blocks:
        for inst in blk.instructions:
            si = inst.sync_info
            if si is None:
                continue
            for u in si.on_update:
                updaters.setdefault(u.id, set()).add(inst.engine)
    for blk in nc.main_func.blocks:
        keep = []
        for inst in blk.instructions:
            si = inst.sync_info
            if si is not None and si.on_wait:
                kept_waits = [
                    w for w in si.on_wait
                    if not (w.sync_type == "semaphore"
                            and updaters.get(w.id) == {inst.engine})
                ]
                if len(kept_waits) != len(si.on_wait):
                    si.on_wait = kept_waits
                if (not kept_waits and not si.on_update
                        and isinstance(inst, mybir.InstEventSemaphore)):
                    continue
            keep.append(inst)
        if len(keep) != len(blk.instructions):
            blk.instructions[:] = keep


def _dedup_ldweights(nc):
    """Drop redundant stationary reloads. Every matmul shares the one ehatT
    stationary, but bass emits an InstLdweights per matmul. A load must stay
    only when it is the first one, carries sync itself, or its matmul has
    more than one wait (Bacc.move_matmul_waits_to_ldweights moves the extra
    wait onto the immediately preceding load). All other loads are deleted,
    leaving the PE stationary resident across steps."""
    import concourse.mybir as mybir

    first = True
    for blk in nc.main_func.blocks:
        insts = blk.instructions
        out = []
        changed = False
        k = 0
        while k < len(insts):
            inst = insts[k]
            if (isinstance(inst, mybir.InstLdweights)
                    and k + 1 < len(insts)
                    and isinstance(insts[k + 1], mybir.InstMatmult)):
                mm = insts[k + 1]
                si = inst.sync_info
                clean = si is None or (not si.on_wait and not si.on_update)
                mmsi = mm.sync_info
                mm_waits = 0 if mmsi is None else len(mmsi.on_wait)
                if clean and mm_waits <= 1 and not first:
                    out.append(mm)
                    changed = True
                else:
                    out.append(inst)
                    out.append(mm)
                first = False
                k += 2
                continue
            out.append(inst)
            k += 1
        if changed:
            blk.instructions[:] = out


def _get_program():
    if "p" not in _PROG_CACHE:
        _PROG_CACHE["p"] = _build_program()
    return _PROG_CACHE["p"]


def _host_prep(logits, trans):
    Ep = np.exp(trans.astype(np.float64)).astype(np.float32)   # [K,K]
    r = np.exp(trans[STOP].astype(np.float64)).astype(np.float32)
    eh = np.zeros((KK, MO), np.float32)        # [contract k, out row m]
    eh[0:K, 0:K] = Ep.T
    eh[K:KK, K:KK] = Ep.T
    eh[0:K, KK] = r
    eh[K:KK, KK + 1] = r
    ehatT = eh.astype(BF16)
    winit = np.zeros((KK, HALF), np.float32)
    winit[START] = 1.0
    winit[K + START] = 1.0
    winit = winit.astype(BF16)
    ex = np.exp(logits.astype(np.float32) - C0)               # [B,T,K]
    in_maps = []
    for c in range(NCORES):
        sh = ex[c * BPC:(c + 1) * BPC]                        # [128,T,K]
        b0 = sh[0:HALF].transpose(2, 1, 0)                    # [K,T,64]
        b1 = sh[HALF:BPC].transpose(2, 1, 0)
        exL = np.ascontiguousarray(
            np.concatenate([b0, b1], axis=0)).astype(BF16)    # [104,T,64]
        in_maps.append({"exL": exL, "ehatT": ehatT, "winit": winit})
    return in_maps


def _host_post(results, lens):
    """partition(t) = ln q_t + C0*t, selected at t = lens[b]."""
    partition = np.empty(B, np.float64)
    tt = np.arange(1, T + 1)
    for c in range(NCORES):
        for g, (lo, wg) in enumerate(CHAINS):
            qarr = results[c][f"qout{g}"].reshape(NBLK, QNR, NS, wg)
            q0 = qarr[:, 8].reshape(NBLK * NS, wg)[:T + 1]    # [513,wg]
            q1 = qarr[:, 9].reshape(NBLK * NS, wg)[:T + 1]
            for qs, boff in ((q0, 0), (q1, HALF)):
                part_at = np.log(qs[1:].astype(np.float64)) + C0 * tt[:, None]
                bidx = c * BPC + boff + lo + np.arange(wg)
                partition[bidx] = part_at[lens[bidx] - 1, np.arange(wg)]
    return partition


def _gold_scores(logits, trans, labels, lens):
    logits64 = logits.astype(np.float64)
    trans64 = trans.astype(np.float64)
    labels_ext = np.concatenate(
        [np.full((B, 1), START, np.int64), labels,
         np.full((B, 1), STOP, np.int64)], 1)
    pos = np.arange(T + 2)[None, :]
    labels_ext = np.where(pos < (lens + 1)[:, None], labels_ext, STOP)
    prev, nxt = labels_ext[:, :-1], labels_ext[:, 1:]
    m_trn = (np.arange(T + 1)[None, :] < (lens + 1)[:, None])
    transition_score = (trans64[nxt, prev] * m_trn).sum(1)
    em = np.take_along_axis(logits64, labels[:, :, None], 2)[:, :, 0]
    m_em = (np.arange(T)[None, :] < lens[:, None])
    emission_score = (em * m_em).sum(1)
    return emission_score, transition_score


def kernel(logits, transitions, labels, lens, _trace=False, **_kw):
    from concourse.bass_utils import run_bass_kernel_spmd

    logits = np.asarray(logits, dtype=np.float32)
    transitions = np.asarray(transitions, dtype=np.float32)
    labels_np = np.asarray(labels).astype(np.int64)
    lens_np = np.asarray(lens).astype(np.int64)

    nc = _get_program()
    in_maps = _host_prep(logits, transitions)
    out = run_bass_kernel_spmd(nc, in_maps, list(range(NCORES)),
                               trace=_trace)
    partition = _host_post(out.results, lens_np)
    emission, transition = _gold_scores(logits, transitions, labels_np,
                                        lens_np)
    loss = partition + emission - transition
    if _trace:
        kernel._last_exec_ns = out.exec_time_ns
        kernel._last_profile = out.profile_json
        kernel._last_out = out
    return loss.astype(np.float32)


# revision 13
# speedup vs baseline: 1.3729x; 1.1936x over previous
"""CRF negative-log-likelihood loss on 8 Trainium2 NeuronCores.

Strategy: data-parallel over batch (128 sequences per core). The forward
(log-partition) recurrence runs on device in the exp domain:

    W_{t+1} = (E' @ W_t) * exp(logits_t - C0),   E' = exp(transitions)

All matmuls are bf16 (1 PE cycle/row vs 4 for fp32). Two batch halves are
stacked block-diagonally on the partition axis (2x52 = 104 contract rows,
106 output rows: 2x52 W' blocks + 2 q rows from r = exp(transitions[STOP])),
so one matmul + one DVE multiply advance 2 batch elements per column.
Three independent chains (22/21/21 columns) hide the PE->DVE->PE latency.
With C0 = 4.9 the state drift over 512 steps stays within fp32/bf16 range,
so no rescaling is needed; the host reconstructs the log-partition as
ln q_t + C0*t and selects t = lens[b]. q rows are copied out of PSUM by the
(otherwise idle) Activation engine and DMA'd per 128-step strip. Gold-path
scores are cheap host-side gathers.
"""

import numpy as np
import ml_dtypes

BF16 = ml_dtypes.bfloat16

# problem constants (hardcoded per contract)
B, T, K = 1024, 512, 52
START, STOP = 50, 51
NCORES = 8
BPC = B // NCORES          # 128 sequences per core
HALF = 64                  # stacked pair columns per core
C0 = 4.9                   # per-step constant log-shift folded into exp(logits)
KK = 2 * K                 # 104 contract rows
MO = KK + 2                # 106 matmul output rows (104 W' + 2 q)
QROW = 96                  # legal AP start partition covering q rows 104..105
QNR = MO - QROW            # 10 rows in the q copy
CH = 64                    # time steps per ex DMA chunk
QB = 8                     # steps per PSUM tile (one 2KB PSUM bank)
NS = 128                   # steps per q strip
NBLK = (T + 1 + NS - 1) // NS   # 5 strips (t = 0..512)
CHAINS = [(0, 32), (32, 32)]   # (col offset, width)
JUNK = 0                   # PE warm-up matmuls per step (p-state hold)

_PROG_CACHE = {}


def _build_program():
    import concourse.mybir as mybir
    import concourse.tile as tile
    from concourse import bacc

    f32 = mybir.dt.float32
    bf16 = mybir.dt.bfloat16

    nc = bacc.Bacc("TRN2", target_bir_lowering=False, debug=False,
                   num_devices=NCORES)
    exL_d = nc.dram_tensor("exL", [KK, T, HALF], bf16, kind="ExternalInput")
    ehatT_d = nc.dram_tensor("ehatT", [KK, MO], bf16, kind="ExternalInput")
    winit_d = nc.dram_tensor("winit", [KK, HALF], bf16, kind="ExternalInput")
    qout_d = [nc.dram_tensor(f"qout{g}", [NBLK, QNR, NS * wg], f32,
                             kind="ExternalOutput")
              for g, (_, wg) in enumerate(CHAINS)]

    with tile.TileContext(nc) as tc:
        with (
            tc.tile_pool(name="const", bufs=1) as cpool,
            tc.tile_pool(name="ex", bufs=2) as expool,
            tc.tile_pool(name="w", bufs=6) as wpool,
            tc.tile_pool(name="qs", bufs=2) as qspool,
            tc.tile_pool(name="u", bufs=3, space="PSUM") as ppool,
            tc.tile_pool(name="uj", bufs=1, space="PSUM") as jpool,
        ):
            ehatT = cpool.tile([KK, MO], bf16)
            nc.sync.dma_start(ehatT[:], ehatT_d[:])
            wjunk = cpool.tile([KK, HALF], bf16)
            nc.sync.dma_start(wjunk[:], winit_d[:])
            ujunk = jpool.tile([MO, HALF], f32, tag="junk", name="ujunk")

            W = []
            for g, (lo, wg) in enumerate(CHAINS):
                w0 = wpool.tile([KK, wg], bf16, tag=f"w{g}", name=f"w0_{g}")
                nc.sync.dma_start(w0[:], winit_d[:, lo:lo + wg])
                W.append(w0)

            ex = None
            ua = [None] * len(CHAINS)
            qsb = [None] * len(CHAINS)
            for t in range(T + 1):
                if t % CH == 0 and t < T:
                    ex = expool.tile([KK, CH, HALF], bf16, tag="ex", name="ex")
                    nc.sync.dma_start(ex[:], exL_d[:, t:t + CH, :])
                i = t % QB
                for g, (lo, wg) in enumerate(CHAINS):
                    if t % NS == 0:
                        qsb[g] = qspool.tile([QNR, NS * wg], f32,
                                             tag=f"qs{g}", name=f"qs{g}")
                    if i == 0:
                        ua[g] = ppool.tile([MO, QB * wg], f32,
                                           tag=f"u{g}", name=f"u{g}")
                    u = ua[g]
                    # u[:, i] = [E'@W0 ; E'@W1 ; q0 ; q1]  (one bf16 matmul)
                    nc.tensor.matmul(u[:, i * wg:(i + 1) * wg], ehatT[:],
                                     W[g][:], start=True, stop=True)
                    if t < T:
                        wn = wpool.tile([KK, wg], bf16, tag=f"w{g}",
                                        name=f"wn{g}")
                        nc.vector.tensor_mul(
                            wn[:], u[0:KK, i * wg:(i + 1) * wg],
                            ex[:, t % CH, lo:lo + wg])
                        W[g] = wn
                    if i == QB - 1 or t == T:
                        # flush q rows (plus the 96..103 pad forced by the
                        # AP start-partition rule) to the SBUF strip
                        n = (i + 1) * wg
                        off = (t - i) % NS * wg
                        nc.scalar.copy(qsb[g][:, off:off + n],
                                       u[QROW:MO, 0:n])
                    if t % NS == NS - 1 or t == T:
                        nc.sync.dma_start(qout_d[g][t // NS], qsb[g][:])
                for _ in range(JUNK if t < T else 0):
                    # dependency-free warmers: keep the PE p-state up so the
                    # latency-critical matmuls run at full clock
                    nc.tensor.matmul(ujunk[:], ehatT[:], wjunk[:],
                                     start=True, stop=True)

    _strip_self_waits(nc)
    _dedup_ldweights(nc)
    nc.compile()
    return nc


def _strip_self_waits(nc):
    """Remove semaphore waits that an engine places on a semaphore updated
    only by that same engine: in-order execution already guarantees them
    (the original program would deadlock otherwise), and each one stalls
    the sequencer for the update-propagation delay of its own previous
    instruction. Standalone InstEventSemaphore carriers left with no waits
    are dropped entirely."""
    import concourse.mybir as mybir

    updaters = {}
    for blk in nc.main_func.blocks:
        for inst in blk.instructions:
            si = inst.sync_info
            if si is None:
                continue
            for u in si.on_update:
                updaters.setdefault(u.id, set()).add(inst.engine)
    for blk in nc.main_func.blocks:
        keep = []
        for inst in blk.instructions:
            si = inst.sync_info
            if si is not None and si.on_wait:
                kept_waits = [
                    w for w in si.on_wait
                    if not (w.sync_type == "semaphore"
                            and updaters.get(w.id) == {inst.engine})
                ]
                if len(kept_waits) != len(si.on_wait):
                    si.on_wait = kept_waits
                if (not kept_waits and not si.on_update
                        and isinstance(inst, mybir.InstEventSemaphore)):
                    continue
            keep.append(inst)
        if len(keep) != len(blk.instructions):
            blk.instructions[:] = keep


def _dedup_ldweights(nc):
    """Drop redundant stationary reloads. Every matmul shares the one ehatT
    stationary, but bass emits an InstLdweights per matmul. A load must stay
    only when it is the first one, carries sync itself, or its matmul has
    more than one wait (Bacc.move_matmul_waits_to_ldweights moves the extra
    wait onto the immediately preceding load). All other loads are deleted,
    leaving the PE stationary resident across steps."""
    import concourse.mybir as mybir

    first = True
    for blk in nc.main_func.blocks:
        insts = blk.instructions
        out = []
        changed = False
        k = 0
        while k < len(insts):
            inst = insts[k]
            if (isinstance(inst, mybir.InstLdweights)
                    and k + 1 < len(insts)
                    and isinstance(insts[k + 1], mybir.InstMatmult)):
                mm = insts[k + 1]
                si = inst.sync_info
                clean = si is None or (not si.on_wait and not si.on_update)
                mmsi = mm.sync_info
                mm_waits = 0 if mmsi is None else len(mmsi.on_wait)
                if clean and mm_waits <= 1 and not first:
                    out.append(mm)
                    changed = True
                else:
                    out.append(inst)
                    out.append(mm)
                first = False
                k += 2
                continue
            out.append(inst)
            k += 1
        if changed:
            blk.instructions[:] = out


def _get_program():
    if "p" not in _PROG_CACHE:
        _PROG_CACHE["p"] = _build_program()
    return _PROG_CACHE["p"]


def _host_prep(logits, trans):
    Ep = np.exp(trans.astype(np.float64)).astype(np.float32)   # [K,K]
    r = np.exp(trans[STOP].astype(np.float64)).astype(np.float32)
    eh = np.zeros((KK, MO), np.float32)        # [contract k, out row m]
    eh[0:K, 0:K] = Ep.T
    eh[K:KK, K:KK] = Ep.T
    eh[0:K, KK] = r
    eh[K:KK, KK + 1] = r
    ehatT = eh.astype(BF16)
    winit = np.zeros((KK, HALF), np.float32)
    winit[START] = 1.0
    winit[K + START] = 1.0
    winit = winit.astype(BF16)
    ex = np.exp(logits.astype(np.float32) - C0)               # [B,T,K]
    in_maps = []
    for c in range(NCORES):
        sh = ex[c * BPC:(c + 1) * BPC]                        # [128,T,K]
        b0 = sh[0:HALF].transpose(2, 1, 0)                    # [K,T,64]
        b1 = sh[HALF:BPC].transpose(2, 1, 0)
        exL = np.ascontiguousarray(
            np.concatenate([b0, b1], axis=0)).astype(BF16)    # [104,T,64]
        in_maps.append({"exL": exL, "ehatT": ehatT, "winit": winit})
    return in_maps


def _host_post(results, lens):
    """partition(t) = ln q_t + C0*t, selected at t = lens[b]."""
    partition = np.empty(B, np.float64)
    tt = np.arange(1, T + 1)
    for c in range(NCORES):
        for g, (lo, wg) in enumerate(CHAINS):
            qarr = results[c][f"qout{g}"].reshape(NBLK, QNR, NS, wg)
            q0 = qarr[:, 8].reshape(NBLK * NS, wg)[:T + 1]    # [513,wg]
            q1 = qarr[:, 9].reshape(NBLK * NS, wg)[:T + 1]
            for qs, boff in ((q0, 0), (q1, HALF)):
                part_at = np.log(qs[1:].astype(np.float64)) + C0 * tt[:, None]
                bidx = c * BPC + boff + lo + np.arange(wg)
                partition[bidx] = part_at[lens[bidx] - 1, np.arange(wg)]
    return partition


def _gold_scores(logits, trans, labels, lens):
    logits64 = logits.astype(np.float64)
    trans64 = trans.astype(np.float64)
    labels_ext = np.concatenate(
        [np.full((B, 1), START, np.int64), labels,
         np.full((B, 1), STOP, np.int64)], 1)
    pos = np.arange(T + 2)[None, :]
    labels_ext = np.where(pos < (lens + 1)[:, None], labels_ext, STOP)
    prev, nxt = labels_ext[:, :-1], labels_ext[:, 1:]
    m_trn = (np.arange(T + 1)[None, :] < (lens + 1)[:, None])
    transition_score = (trans64[nxt, prev] * m_trn).sum(1)
    em = np.take_along_axis(logits64, labels[:, :, None], 2)[:, :, 0]
    m_em = (np.arange(T)[None, :] < lens[:, None])
    emission_score = (em * m_em).sum(1)
    return emission_score, transition_score


def kernel(logits, transitions, labels, lens, _trace=False, **_kw):
    from concourse.bass_utils import run_bass_kernel_spmd

    logits = np.asarray(logits, dtype=np.float32)
    transitions = np.asarray(transitions, dtype=np.float32)
    labels_np = np.asarray(labels).astype(np.int64)
    lens_np = np.asarray(lens).astype(np.int64)

    nc = _get_program()
    in_maps = _host_prep(logits, transitions)
    out = run_bass_kernel_spmd(nc, in_maps, list(range(NCORES)),
                               trace=_trace)
    partition = _host_post(out.results, lens_np)
    emission, transition = _gold_scores(logits, transitions, labels_np,
                                        lens_np)
    loss = partition + emission - transition
    if _trace:
        kernel._last_exec_ns = out.exec_time_ns
        kernel._last_profile = out.profile_json
        kernel._last_out = out
    return loss.astype(np.float32)
